# revision 34
# baseline (speedup 1.0000x reference)
"""AdaptiveGCNLayer Trainium2 kernel (8 NeuronCores, data-parallel over frames).

The reference module's adaptive-adjacency branch is dead code (its result is
never used).  Because edge_index is shared by every frame (offsets just shift
it per frame), the live computation collapses to

    out[f] = M @ x[f] @ gcn_W + gcn_b        for every frame f

with a single 25x25 normalized-adjacency matrix M (PyG GCNConv norm with
self-loops) computed on host from the 48 edges.

Sharding: frames are data-parallel across the 8 cores.  Each core's shard is
packed on host into tile-major layout [125 partitions, 205 tiles, 128 ch]
(5 frames = 125 rows per tile; the ragged tail is zero-padded) so every
HBM<->SBUF DMA is per-partition contiguous.

Device kernel (per core):
  - mm1: T1 = lhsT(x_tile).T @ (I5 (x) M^T)   -> (M5 @ X)^T in PSUM (no transposes)
  - copy T1 -> SBUF
  - mm2: O = lhsT(T1).T @ W                   -> natural row-major output in PSUM
  - DVE adds bias while copying PSUM -> SBUF
  - big contiguous HWDGE DMAs in and out

Two compute modes (KERNEL_MODE env): "bf16" casts x to bf16 on ACT/DVE and
runs bf16 matmuls; "f32r" feeds fp32 bits straight to the PE as float32r
with the moving free dim padded to 256 (full-rate per the cost model).
"""

import os
import numpy as np
import ml_dtypes

B, V, C = 8192, 25, 128
NCORES = 8
FRAMES_PER_CORE = B // NCORES          # 1024
ROWS = FRAMES_PER_CORE * V             # 25600
FPT = 5                                # frames per matmul tile
TROWS = FPT * V                        # 125 rows per tile
NT = 205                               # tiles per core (last one padded)
FULL_T = ROWS // TROWS                 # 204 full tiles
TAIL_ROWS = ROWS - FULL_T * TROWS      # 100
# variable group sizes: a small final group keeps the kernel-tail output
# transfer (which nothing overlaps) tiny
GROUPS = [(0, 48), (48, 48), (96, 48), (144, 48), (192, 13)]
TPG = max(sz for _, sz in GROUPS)      # SBUF slot size
JB = 4                                 # tiles per PSUM batch (1 PSUM bank)
MODE = os.environ.get("KERNEL_MODE", "bf16")
NF32R = 256                            # padded moving free dim for f32r

_CACHE = {}


def _build_graph(with_bias=True):
    import concourse.mybir as mybir
    import concourse.tile as tile
    from concourse import bacc

    f32 = mybir.dt.float32
    bf16 = mybir.dt.bfloat16

    nc = bacc.Bacc("TRN2", target_bir_lowering=False, debug=False,
                   num_devices=NCORES)

    # x arrives pre-cast to bf16 by the host (halves input traffic, no
    # on-chip cast stage needed)
    x_in = nc.declare_dram_parameter("x", [TROWS, NT, C], bf16, isOutput=False)
    m5t_in = nc.declare_dram_parameter("m5t", [TROWS, C], bf16, isOutput=False)
    w_in = nc.declare_dram_parameter("w", [C, C], bf16, isOutput=False)
    if with_bias:
        b_in = nc.declare_dram_parameter("bias", [TROWS, JB, C], f32,
                                         isOutput=False)
    # output in bf16 (host upcasts back to f32) — halves output traffic
    out_ext = nc.declare_dram_parameter("out", [TROWS, NT, C], bf16, isOutput=True)

    with tile.TileContext(nc) as tc:
        with (
            tc.tile_pool(name="consts", bufs=1) as consts,
            tc.tile_pool(name="xp", bufs=5) as xp,
            tc.tile_pool(name="op", bufs=3) as op_pool,
            tc.tile_pool(name="t1s", bufs=3) as t1sp,
            tc.tile_pool(name="t1psum", bufs=2, space=tile.bass.MemorySpace.PSUM) as t1pp,
            tc.tile_pool(name="opsum", bufs=2, space=tile.bass.MemorySpace.PSUM) as opp,
        ):
            m5t_sb = consts.tile([TROWS, C], bf16)
            w_sb = consts.tile([C, C], bf16)
            nc.sync.dma_start(out=m5t_sb[:], in_=m5t_in[:])
            nc.sync.dma_start(out=w_sb[:], in_=w_in[:])
            if with_bias:
                bias_sb = consts.tile([TROWS, JB, C], f32)
                nc.sync.dma_start(out=bias_sb[:], in_=b_in[:])

            def in_slices(g, gsz):
                if g == 0:
                    return ((0, 4), (4, 12), (16, 16), (32, 16))
                if gsz == 48:
                    return ((0, 16), (16, 16), (32, 16))
                return ((0, gsz),)

            def out_slices(g, gsz):
                if gsz == 48:
                    return ((0, 16), (16, 16), (32, 16))
                return ((0, 5), (5, 4), (9, 4))

            for g, (t0, gsz) in enumerate(GROUPS):
                x_t = xp.tile([128, TPG, C], bf16, tag="x")
                # SWDGE loads (cheap async triggers), contiguous per
                # partition, bf16; sliced so the first matmuls unblock
                # before the whole group lands
                for s0, sn in in_slices(g, gsz):
                    nc.gpsimd.dma_start(out=x_t[0:TROWS, s0:s0 + sn, :],
                                        in_=x_in[:, t0 + s0:t0 + s0 + sn, :])

                o_t = op_pool.tile([128, TPG, C], bf16, tag="o")

                for bi, j0 in enumerate(range(0, gsz, JB)):
                    nb = min(JB, gsz - j0)
                    t1p = t1pp.tile([128, JB, C], f32, tag="t1p")
                    for u in range(nb):
                        nc.tensor.matmul(t1p[:, u, :],
                                         lhsT=x_t[0:TROWS, j0 + u, :],
                                         rhs=m5t_sb[:, :],
                                         start=True, stop=True)
                    t1s = t1sp.tile([128, JB, C], bf16, tag="t1s")
                    if with_bias:
                        # ACT (1-input) does the T1 copy, DVE the bias add
                        nc.scalar.copy(t1s[:, 0:nb, :], t1p[:, 0:nb, :])
                    elif bi % 2 == 0:
                        # no bias: both PSUM->SBUF moves are plain copies —
                        # alternate engines so the PE-feeding T1 copy is not
                        # always queued behind one engine's backlog
                        nc.scalar.copy(t1s[:, 0:nb, :], t1p[:, 0:nb, :])
                    else:
                        nc.vector.tensor_copy(t1s[:, 0:nb, :], t1p[:, 0:nb, :])
                    o_ps = opp.tile([128, JB, C], f32, tag="ops")
                    for u in range(nb):
                        nc.tensor.matmul(o_ps[:, u, :],
                                         lhsT=t1s[:, u, :],
                                         rhs=w_sb[:, :],
                                         start=True, stop=True)
                    if with_bias:
                        nc.vector.tensor_add(o_t[0:TROWS, j0:j0 + nb, :],
                                             o_ps[0:TROWS, 0:nb, :],
                                             bias_sb[:, 0:nb, :])
                    elif bi % 2 == 0:
                        nc.vector.tensor_copy(o_t[0:TROWS, j0:j0 + nb, :],
                                              o_ps[0:TROWS, 0:nb, :])
                    else:
                        nc.scalar.copy(o_t[0:TROWS, j0:j0 + nb, :],
                                       o_ps[0:TROWS, 0:nb, :])

                # output also SWDGE, sliced
                for s0, sn in out_slices(g, gsz):
                    nc.gpsimd.dma_start(
                        out=out_ext[:, t0 + s0:t0 + s0 + sn, :],
                        in_=o_t[0:TROWS, s0:s0 + sn, :])

    nc.compile()
    return nc


def _get_graph(with_bias):
    key = ("nc", with_bias)
    if key not in _CACHE:
        _CACHE[key] = _build_graph(with_bias)
    return _CACHE[key]


def _host_prep(edge_index, gcn_W, gcn_b):
    ei = np.asarray(edge_index).astype(np.int64)
    rows, cols = ei[0], ei[1]
    deg = np.bincount(cols, minlength=V).astype(np.float32) + 1.0  # + self loop
    dis = (1.0 / np.sqrt(deg)).astype(np.float32)
    M = np.zeros((V, V), np.float32)
    np.add.at(M, (cols, rows), dis[rows] * dis[cols])
    M[np.arange(V), np.arange(V)] += dis * dis
    m5t_pad = np.zeros((TROWS, C), np.float32)
    m5t_pad[:, :TROWS] = np.kron(np.eye(FPT, dtype=np.float32), M.T)
    bias_t = np.ascontiguousarray(
        np.broadcast_to(np.asarray(gcn_b, np.float32), (TROWS, JB, C)))
    return (m5t_pad.astype(ml_dtypes.bfloat16),
            np.asarray(gcn_W, np.float32).astype(ml_dtypes.bfloat16),
            bias_t)


def _pack(x):
    """(B, V, C) f32 -> per-core tile-major bf16 [NCORES, TROWS, NT, C]."""
    xr = np.asarray(x, np.float32).reshape(NCORES, ROWS, C)
    packed = np.zeros((NCORES, NT, TROWS, C), np.float32)
    packed[:, :FULL_T] = xr[:, :FULL_T * TROWS].reshape(NCORES, FULL_T, TROWS, C)
    packed[:, FULL_T, :TAIL_ROWS] = xr[:, FULL_T * TROWS:]
    return np.ascontiguousarray(
        packed.transpose(0, 2, 1, 3).astype(ml_dtypes.bfloat16))


def _unpack(outs):
    """[NCORES, TROWS, NT, C] (bf16) -> (B, V, C) f32."""
    o = outs.transpose(0, 2, 1, 3).astype(np.float32)  # [NCORES, NT, TROWS, C]
    res = np.empty((NCORES, ROWS, C), np.float32)
    res[:, :FULL_T * TROWS] = o[:, :FULL_T].reshape(NCORES, FULL_T * TROWS, C)
    res[:, FULL_T * TROWS:] = o[:, FULL_T, :TAIL_ROWS]
    return res.reshape(B, V, C)


def kernel(x, edge_index, adj_matrix=None, aw_W=None, aw_b=None,
           gcn_W=None, gcn_b=None, **_unused):
    from concourse.bass_utils import run_bass_kernel_spmd

    m5t_h, w_h, bias_t = _host_prep(edge_index, gcn_W, gcn_b)
    with_bias = bool(np.any(np.asarray(gcn_b, np.float32)))
    xp = _pack(x)
    in_maps = []
    for i in range(NCORES):
        m = {"x": xp[i], "m5t": m5t_h, "w": w_h}
        if with_bias:
            m["bias"] = bias_t
        in_maps.append(m)
    res = run_bass_kernel_spmd(_get_graph(with_bias), in_maps,
                               core_ids=list(range(NCORES)))
    out = np.stack([r["out"] for r in res.results])
    return _unpack(out)
